# revision 10
# baseline (speedup 1.0000x reference)
"""Trainium2 Bass kernel for nn_ChannelSpatialModulatedConv2d.

Data-parallel over batch across 8 NeuronCores (4 samples each).

All batch-parallel small math (EqualLinear style, weight demod, spatial map
+ its demod) runs on HOST in float64 — a few MFLOPs. The per-sample
modulated+demodulated weights are also folded on host:

  wsc[b, ci, co*kk] = weight[co,ci,kk] * style[b,ci] * (CS*demod[b,co]*demod_sp[b])

so the device kernel is a pure bf16 conv stream:

  per (b, m, n) output tile [128co x 512yx]: 18 accumulating bf16 matmuls
     (2 ci-tiles x 9 taps) over a zero-padded 66x66 image, shifted-window APs
  epilogue (DVE): out = psum * spmap[yx]   (spmap = raw sp map, bf16,
     broadcast 1->128 partitions by DMA)

PE stream = ~40 warm-up matmuls (HAM K=8/8 by the time real MMs start)
followed by 1152 conv matmuls at the N=512 streaming roofline. The only
other device compute is the 64 epilogue multiplies on DVE.

DMA: sync(HWDGE) = x bands + outs (first x band thinned to rows 0-10);
scalar(HWDGE) = wsc tiles (sample 0 split into m-halves so conv starts
ASAP) + spmap broadcasts (sample 0 up-front, rest staggered per sample).

The baked walrus build only supports ONE sync wait per instruction, so the
Bass subclass rewrites the scheduled BIR JSON, hoisting extra waits onto
single-wait EventSemaphore carriers inserted before the instruction (same
engine => identical blocking semantics).
"""

import json
import sys
from contextlib import ExitStack

for _p in ("/opt/pypackages", "/opt/trn_rl_repo"):
    if _p not in sys.path:
        sys.path.insert(0, _p)

import ml_dtypes
import numpy as np

import concourse.bass as bass
import concourse.mybir as mybir
import concourse.tile as tile
from concourse.bass_utils import run_bass_kernel_spmd

# Problem constants (hardcoded per harness contract)
B, CIN, COUT, K = 32, 256, 256, 3
STYLE_DIM, SP = 512, 64
EPS = 1e-6
LS = 1.0 / (STYLE_DIM // 2) ** 0.5      # EqualLinear scale = 1/16
CS = 1.0 / (CIN * K * K) ** 0.5         # conv fan-in scale = 1/48
N_CORES = 8
BPC = B // N_CORES                      # samples per core = 4
SPP = SP + 2                            # padded image dim = 66
CKK = COUT * K * K                      # 2304 free columns in weight layout
YX = SP * SP                            # 4096 spatial positions
HALF = (CKK // 2)                       # m=0 half of the weight free dim

F32 = mybir.dt.float32
BF16 = mybir.dt.bfloat16
ALU = mybir.AluOpType

# x row bands per ci-tile: band i covers padded rows [lo, hi)
XBANDS0 = [(0, 10), (10, 34), (34, 66)]   # sample 0: thin first band
XBANDS = [(0, 18), (18, 42), (42, 66)]    # prefetched samples
N_WARM = 40                               # HAM warm-up matmuls


def _split_multi_waits(bir: dict) -> int:
    """Hoist all but one sync wait from every instruction onto single-wait
    EventSemaphore carriers inserted immediately before it (same engine)."""
    ctr = 0
    for fn in bir.get("functions", []):
        for blk in fn.get("blocks", []):
            insts = blk.get("instructions", [])
            if not any(
                len(((i.get("sync_info") or {}).get("on_wait") or [])) > 1
                for i in insts
            ):
                continue
            new_insts = []
            for inst in insts:
                si = inst.get("sync_info")
                ow = (si or {}).get("on_wait") or []
                if len(ow) > 1:
                    for w in ow[:-1]:
                        ctr += 1
                        new_insts.append({
                            "debug": inst.get("debug", 0),
                            "engine": inst["engine"],
                            "ins": [],
                            "outs": [],
                            "name": f"waitsplit-{ctr}",
                            "opcode": "EventSemaphore",
                            "sync_info": {"on_update": [], "on_wait": [w]},
                        })
                    si["on_wait"] = [ow[-1]]
                new_insts.append(inst)
            blk["instructions"] = new_insts
    return ctr


class _WaitSplitBass(bass.Bass):
    def to_json_bytes(self) -> bytes:
        raw = super().to_json_bytes()
        bir = json.loads(raw)
        if _split_multi_waits(bir):
            return json.dumps(bir).encode()
        return raw


def _pbcast(ap, n):
    """Manual 0-step partition broadcast AP (DMA-only; engines reject it)."""
    return bass.AP(tensor=ap.tensor, offset=ap.offset,
                   ap=[[0, n]] + [list(d) for d in ap.ap[1:]])


def _build_program() -> bass.Bass:
    nc = _WaitSplitBass("TRN2", target_bir_lowering=False, debug=False)

    x_d = nc.dram_tensor("x", [BPC, CIN, SPP, SPP], BF16, kind="ExternalInput")
    wsc_d = nc.dram_tensor("wsc", [BPC, CIN, CKK], BF16, kind="ExternalInput")
    spmd_d = nc.dram_tensor("spmd", [BPC, YX], BF16, kind="ExternalInput")
    out_d = nc.dram_tensor("out", [BPC, COUT, SP, SP], F32, kind="ExternalOutput")

    with tile.TileContext(nc) as tc:
        with tc.tile_pool(name="const", bufs=1) as cpool, \
             tc.tile_pool(name="warm", bufs=1, space="PSUM") as wpsum:

            warm_w = cpool.tile([128, 128], BF16, name="warm_w")

            _stack = ExitStack()
            xppool = _stack.enter_context(tc.tile_pool(name="xp", bufs=2))
            cpsum = _stack.enter_context(tc.tile_pool(name="cps", bufs=7, space="PSUM"))
            wscpool = _stack.enter_context(tc.tile_pool(name="wsc", bufs=2))
            opool = _stack.enter_context(tc.tile_pool(name="ot", bufs=8))
            smpool = _stack.enter_context(tc.tile_pool(name="smb", bufs=32))

            # ---------- warm-up matmuls: keep the PE busy from t~7us so the
            # HAM clock gate is at K=8/8 (2.4 GHz) before real matmuls.
            # The k=1 m=0-half weight load rides the tensor-engine DMA queue,
            # issued between early warm-ups (parallel DGE ramp with scalar). --
            nc.vector.memset(warm_w, 0.0)
            scratch = wpsum.tile([128, 128], F32, name="scratch")
            wsc_tiles = [None] * BPC
            ws0 = [
                wscpool.tile([128, CKK], BF16, name=f"wsc{k}_0", tag=f"wsc{k}")
                for k in range(2)
            ]
            wsc_tiles[0] = ws0
            for i in range(N_WARM):
                nc.tensor.matmul(scratch, warm_w, warm_w, start=True, stop=True)

            # ---------- wsc loads (scalar queue; sample 0 m-split) ----------
            def load_wsc(b, split=False):
                if split:
                    # k=0 m=0-half on sync (ahead of the x bands there), the
                    # rest on scalar — the two queues ramp in parallel
                    ws = wsc_tiles[b]
                    nc.sync.dma_start(
                        out=ws[0][:, 0:HALF],
                        in_=wsc_d.ap()[b, 0:128, 0:HALF])
                    nc.scalar.dma_start(
                        out=ws[1][:, 0:HALF],
                        in_=wsc_d.ap()[b, 128:256, 0:HALF])
                    for k in range(2):
                        nc.scalar.dma_start(
                            out=ws[k][:, HALF:CKK],
                            in_=wsc_d.ap()[b, k * 128:(k + 1) * 128, HALF:CKK])
                else:
                    ws = [
                        wscpool.tile([128, CKK], BF16, name=f"wsc{k}_{b}", tag=f"wsc{k}")
                        for k in range(2)
                    ]
                    for k in range(2):
                        nc.scalar.dma_start(
                            out=ws[k],
                            in_=wsc_d.ap()[b, k * 128:(k + 1) * 128, :])
                    wsc_tiles[b] = ws

            smb_tiles = [[None] * 8 for _ in range(BPC)]

            def load_smb(b):
                for n in range(8):
                    t = smpool.tile([128, 512], BF16, name=f"smb_{b}_{n}", tag="smb")
                    nc.scalar.dma_start(
                        out=t,
                        in_=_pbcast(spmd_d.ap()[b:b + 1, n * 512:(n + 1) * 512], 128),
                    )
                    smb_tiles[b][n] = t

            load_wsc(0, split=True)
            load_smb(0)

            # ---------- x band loads (sync queue) ----------
            xp0 = [
                xppool.tile([128, SPP * SPP], BF16, name=f"xp{k}_0", tag=f"xp{k}")
                for k in range(2)
            ]
            xp_tiles = [xp0] + [None] * (BPC - 1)

            def load_band(xp, b, bi, k, bands=XBANDS):
                lo, hi = bands[bi]
                nc.sync.dma_start(
                    out=xp[k][:, lo * SPP:hi * SPP],
                    in_=x_d.ap()[b, k * 128:(k + 1) * 128, lo:hi, :]
                        .rearrange("p r c -> p (r c)"),
                )

            for bi in range(3):
                load_band(xp0, 0, bi, 0, XBANDS0)
                load_band(xp0, 0, bi, 1, XBANDS0)

            def prefetch_xp(b):
                xp = [
                    xppool.tile([128, SPP * SPP], BF16, name=f"xp{k}_{b}", tag=f"xp{k}")
                    for k in range(2)
                ]
                xp_tiles[b] = xp
                return xp

            # ---------- per-sample conv pipeline ----------
            for b in range(BPC):
                wsc = wsc_tiles[b]
                xp = xp_tiles[b]
                prefetched = False

                for m in range(2):
                    for n in range(8):
                        ps = cpsum.tile([128, 512], F32, name=f"ps_{b}_{m}_{n}", tag="ps")
                        i = 0
                        for k in range(2):
                            wv = wsc[k].rearrange("p (co kk) -> p co kk", kk=9)
                            xpv = xp[k].rearrange("p (r c) -> p r c", c=SPP)
                            for s in range(9):
                                dy, dx = s // 3, s % 3
                                nc.tensor.matmul(
                                    ps,
                                    wv[:, m * 128:(m + 1) * 128, s],
                                    xpv[:, n * 8 + dy:n * 8 + dy + 8, dx:dx + SP],
                                    start=(i == 0), stop=(i == 17),
                                )
                                i += 1
                        # prefetch next sample's weights + spmap broadcasts
                        if m == 0 and n == 6 and b + 1 < BPC:
                            load_wsc(b + 1)
                            load_smb(b + 1)
                        # prefetch next sample's image in m=1
                        if m == 1 and 1 <= n <= 3 and b + 1 < BPC:
                            if not prefetched:
                                xpn = prefetch_xp(b + 1)
                                prefetched = True
                            load_band(xpn, b + 1, n - 1, 0)
                            load_band(xpn, b + 1, n - 1, 1)
                        # epilogue: out = psum * spmap[yx] (all scales folded
                        # into wsc on host)
                        ot = opool.tile([128, 512], F32,
                                        name=f"ot_{b}_{m}_{n}", tag="ot")
                        nc.vector.tensor_tensor(
                            out=ot, in0=ps,
                            in1=smb_tiles[b][n], op=ALU.mult,
                        )
                        nc.sync.dma_start(
                            out=out_d.ap()[b, m * 128:(m + 1) * 128, n * 8:(n + 1) * 8, :],
                            in_=ot.rearrange("p (r c) -> p r c", c=SP),
                        )
            _stack.close()
    return nc


_prog_cache = {}


def _get_program() -> bass.Bass:
    if "nc" not in _prog_cache:
        _prog_cache["nc"] = _build_program()
    return _prog_cache["nc"]


def _make_in_maps(inputs):
    x = np.asarray(inputs["x"], dtype=np.float32)
    x = np.pad(x, ((0, 0), (0, 0), (1, 1), (1, 1))).astype(ml_dtypes.bfloat16)
    style_in = np.asarray(inputs["style_in"], dtype=np.float64)
    weight = np.asarray(inputs["weight"], dtype=np.float64)
    mod_w = np.asarray(inputs["mod_w"], dtype=np.float64)
    mod_b = np.asarray(inputs["mod_b"], dtype=np.float64)
    sp_w = np.asarray(inputs["sp_w"], dtype=np.float64)
    sp_b = np.asarray(inputs["sp_b"], dtype=np.float64)

    # ---- host-side small math (float64, a few MFLOPs total) ----
    style_chan, style_spatial = style_in[:, :256], style_in[:, 256:]
    style = style_chan @ (mod_w * LS).T + mod_b                 # [B, CIN]
    # demod via S2q[co,ci] = sum_kk weight^2 (exact same sum as reference)
    w0 = weight[0]                                              # [COUT,CIN,3,3]
    s2q = np.sum(w0 * w0, axis=(2, 3))                          # [COUT, CIN]
    demodsq = (CS * CS) * (style * style) @ s2q.T               # [B, COUT]
    demod = 1.0 / np.sqrt(demodsq + EPS)
    sp = style_spatial @ (sp_w * LS).T + sp_b                   # [B, YX]
    demod_sp = np.sqrt(YX / np.sum(sp * sp, axis=1) + EPS)      # [B]
    dcol = CS * demod * demod_sp[:, None]                       # [B, COUT]
    spmd = sp.astype(ml_dtypes.bfloat16)                        # [B, YX]

    # per-sample folded weights: [B, CIN, COUT*KK] bf16
    wT = np.ascontiguousarray(w0.transpose(1, 0, 2, 3)).reshape(CIN, COUT, K * K)
    wsc = (wT[None].astype(np.float32)
           * style.astype(np.float32)[:, :, None, None]
           * dcol.astype(np.float32)[:, None, :, None])         # [B,CIN,COUT,KK]
    wsc = wsc.reshape(B, CIN, CKK).astype(ml_dtypes.bfloat16)

    in_maps = []
    for c in range(N_CORES):
        sl = slice(c * BPC, (c + 1) * BPC)
        in_maps.append({
            "x": np.ascontiguousarray(x[sl]),
            "wsc": np.ascontiguousarray(wsc[sl]),
            "spmd": np.ascontiguousarray(spmd[sl]),
        })
    return in_maps


def _run(inputs, trace=False):
    nc = _get_program()
    in_maps = _make_in_maps(inputs)
    res = run_bass_kernel_spmd(nc, in_maps, core_ids=list(range(N_CORES)), trace=trace)
    out = np.concatenate([res.results[c]["out"] for c in range(N_CORES)], axis=0)
    return out, res


def kernel(**inputs) -> np.ndarray:
    out, _ = _run(inputs, trace=False)
    return out


# revision 12
# speedup vs baseline: 1.0045x; 1.0045x over previous
"""Trainium2 Bass kernel for nn_ChannelSpatialModulatedConv2d.

Data-parallel over batch across 8 NeuronCores (4 samples each).

All batch-parallel small math (EqualLinear style, weight demod, spatial map
+ its demod) runs on HOST in float64 — a few MFLOPs. The per-sample
modulated+demodulated weights are also folded on host:

  wsc[b, ci, co*kk] = weight[co,ci,kk] * style[b,ci] * (CS*demod[b,co]*demod_sp[b])

so the device kernel is a pure bf16 conv stream:

  per (b, m, n) output tile [128co x 512yx]: 18 accumulating bf16 matmuls
     (2 ci-tiles x 9 taps) over a zero-padded 66x66 image, shifted-window APs
  epilogue (DVE): out = psum * spmap[yx]   (spmap = raw sp map, bf16,
     broadcast 1->128 partitions by DMA)

PE stream = ~40 warm-up matmuls (HAM K=8/8 by the time real MMs start)
followed by 1152 conv matmuls at the N=512 streaming roofline. The only
other device compute is the 64 epilogue multiplies on DVE.

DMA: sync(HWDGE) = x bands + outs (first x band thinned to rows 0-10);
scalar(HWDGE) = wsc tiles (sample 0 split into m-halves so conv starts
ASAP) + spmap broadcasts (sample 0 up-front, rest staggered per sample).

The baked walrus build only supports ONE sync wait per instruction, so the
Bass subclass rewrites the scheduled BIR JSON, hoisting extra waits onto
single-wait EventSemaphore carriers inserted before the instruction (same
engine => identical blocking semantics).
"""

import json
import sys
from contextlib import ExitStack

for _p in ("/opt/pypackages", "/opt/trn_rl_repo"):
    if _p not in sys.path:
        sys.path.insert(0, _p)

import ml_dtypes
import numpy as np

import concourse.bass as bass
import concourse.mybir as mybir
import concourse.tile as tile
from concourse.bass_utils import run_bass_kernel_spmd

# Problem constants (hardcoded per harness contract)
B, CIN, COUT, K = 32, 256, 256, 3
STYLE_DIM, SP = 512, 64
EPS = 1e-6
LS = 1.0 / (STYLE_DIM // 2) ** 0.5      # EqualLinear scale = 1/16
CS = 1.0 / (CIN * K * K) ** 0.5         # conv fan-in scale = 1/48
N_CORES = 8
BPC = B // N_CORES                      # samples per core = 4
SPP = SP + 2                            # padded image dim = 66
CKK = COUT * K * K                      # 2304 free columns in weight layout
YX = SP * SP                            # 4096 spatial positions
HALF = (CKK // 2)                       # m=0 half of the weight free dim

F32 = mybir.dt.float32
BF16 = mybir.dt.bfloat16
ALU = mybir.AluOpType

# x row bands per ci-tile: band i covers padded rows [lo, hi)
XBANDS0 = [(0, 10), (10, 34), (34, 66)]   # sample 0: thin first band
XBANDS = [(0, 18), (18, 42), (42, 66)]    # prefetched samples
N_WARM = 44                               # HAM warm-up matmuls


def _split_multi_waits(bir: dict) -> int:
    """Hoist all but one sync wait from every instruction onto single-wait
    EventSemaphore carriers inserted immediately before it (same engine)."""
    ctr = 0
    for fn in bir.get("functions", []):
        for blk in fn.get("blocks", []):
            insts = blk.get("instructions", [])
            if not any(
                len(((i.get("sync_info") or {}).get("on_wait") or [])) > 1
                for i in insts
            ):
                continue
            new_insts = []
            for inst in insts:
                si = inst.get("sync_info")
                ow = (si or {}).get("on_wait") or []
                if len(ow) > 1:
                    for w in ow[:-1]:
                        ctr += 1
                        new_insts.append({
                            "debug": inst.get("debug", 0),
                            "engine": inst["engine"],
                            "ins": [],
                            "outs": [],
                            "name": f"waitsplit-{ctr}",
                            "opcode": "EventSemaphore",
                            "sync_info": {"on_update": [], "on_wait": [w]},
                        })
                    si["on_wait"] = [ow[-1]]
                new_insts.append(inst)
            blk["instructions"] = new_insts
    return ctr


class _WaitSplitBass(bass.Bass):
    def to_json_bytes(self) -> bytes:
        raw = super().to_json_bytes()
        bir = json.loads(raw)
        if _split_multi_waits(bir):
            return json.dumps(bir).encode()
        return raw


def _pbcast(ap, n):
    """Manual 0-step partition broadcast AP (DMA-only; engines reject it)."""
    return bass.AP(tensor=ap.tensor, offset=ap.offset,
                   ap=[[0, n]] + [list(d) for d in ap.ap[1:]])


def _build_program() -> bass.Bass:
    nc = _WaitSplitBass("TRN2", target_bir_lowering=False, debug=False)

    x_d = nc.dram_tensor("x", [BPC, CIN, SPP, SPP], BF16, kind="ExternalInput")
    wsc_d = nc.dram_tensor("wsc", [BPC, CIN, CKK], BF16, kind="ExternalInput")
    spmd_d = nc.dram_tensor("spmd", [BPC, YX], BF16, kind="ExternalInput")
    out_d = nc.dram_tensor("out", [BPC, COUT, SP, SP], F32, kind="ExternalOutput")

    with tile.TileContext(nc) as tc:
        with tc.tile_pool(name="const", bufs=1) as cpool, \
             tc.tile_pool(name="warm", bufs=1, space="PSUM") as wpsum:

            warm_w = cpool.tile([128, 128], BF16, name="warm_w")

            _stack = ExitStack()
            xppool = _stack.enter_context(tc.tile_pool(name="xp", bufs=2))
            cpsum = _stack.enter_context(tc.tile_pool(name="cps", bufs=7, space="PSUM"))
            wscpool = _stack.enter_context(tc.tile_pool(name="wsc", bufs=2))
            opool = _stack.enter_context(tc.tile_pool(name="ot", bufs=8))
            smpool = _stack.enter_context(tc.tile_pool(name="smb", bufs=32))

            # ---------- warm-up matmuls: keep the PE busy from t~7us so the
            # HAM clock gate is at K=8/8 (2.4 GHz) before real matmuls.
            # The k=1 m=0-half weight load rides the tensor-engine DMA queue,
            # issued between early warm-ups (parallel DGE ramp with scalar). --
            nc.vector.memset(warm_w, 0.0)
            scratch = wpsum.tile([128, 128], F32, name="scratch")
            wsc_tiles = [None] * BPC
            ws0 = [
                wscpool.tile([128, CKK], BF16, name=f"wsc{k}_0", tag=f"wsc{k}")
                for k in range(2)
            ]
            wsc_tiles[0] = ws0
            for i in range(N_WARM):
                nc.tensor.matmul(scratch, warm_w, warm_w, start=True, stop=True)

            # ---------- wsc loads (scalar queue; sample 0 m-split) ----------
            def load_wsc(b, split=False):
                if split:
                    # all on scalar queue (x bands own sync); m=0 halves first
                    ws = wsc_tiles[b]
                    for k in range(2):
                        nc.scalar.dma_start(
                            out=ws[k][:, 0:HALF],
                            in_=wsc_d.ap()[b, k * 128:(k + 1) * 128, 0:HALF])
                    for k in range(2):
                        nc.scalar.dma_start(
                            out=ws[k][:, HALF:CKK],
                            in_=wsc_d.ap()[b, k * 128:(k + 1) * 128, HALF:CKK])
                else:
                    ws = [
                        wscpool.tile([128, CKK], BF16, name=f"wsc{k}_{b}", tag=f"wsc{k}")
                        for k in range(2)
                    ]
                    for k in range(2):
                        nc.scalar.dma_start(
                            out=ws[k],
                            in_=wsc_d.ap()[b, k * 128:(k + 1) * 128, :])
                    wsc_tiles[b] = ws

            smb_tiles = [[None] * 8 for _ in range(BPC)]

            def load_smb(b):
                for n in range(8):
                    t = smpool.tile([128, 512], BF16, name=f"smb_{b}_{n}", tag="smb")
                    nc.scalar.dma_start(
                        out=t,
                        in_=_pbcast(spmd_d.ap()[b:b + 1, n * 512:(n + 1) * 512], 128),
                    )
                    smb_tiles[b][n] = t

            load_wsc(0, split=True)
            load_smb(0)

            # ---------- x band loads (sync queue) ----------
            xp0 = [
                xppool.tile([128, SPP * SPP], BF16, name=f"xp{k}_0", tag=f"xp{k}")
                for k in range(2)
            ]
            xp_tiles = [xp0] + [None] * (BPC - 1)

            def load_band(xp, b, bi, k, bands=XBANDS):
                lo, hi = bands[bi]
                nc.sync.dma_start(
                    out=xp[k][:, lo * SPP:hi * SPP],
                    in_=x_d.ap()[b, k * 128:(k + 1) * 128, lo:hi, :]
                        .rearrange("p r c -> p (r c)"),
                )

            for bi in range(3):
                load_band(xp0, 0, bi, 0, XBANDS0)
                load_band(xp0, 0, bi, 1, XBANDS0)

            def prefetch_xp(b):
                xp = [
                    xppool.tile([128, SPP * SPP], BF16, name=f"xp{k}_{b}", tag=f"xp{k}")
                    for k in range(2)
                ]
                xp_tiles[b] = xp
                return xp

            # ---------- per-sample conv pipeline ----------
            for b in range(BPC):
                wsc = wsc_tiles[b]
                xp = xp_tiles[b]
                prefetched = False

                for m in range(2):
                    for n in range(8):
                        ps = cpsum.tile([128, 512], F32, name=f"ps_{b}_{m}_{n}", tag="ps")
                        i = 0
                        for k in range(2):
                            wv = wsc[k].rearrange("p (co kk) -> p co kk", kk=9)
                            xpv = xp[k].rearrange("p (r c) -> p r c", c=SPP)
                            for s in range(9):
                                dy, dx = s // 3, s % 3
                                nc.tensor.matmul(
                                    ps,
                                    wv[:, m * 128:(m + 1) * 128, s],
                                    xpv[:, n * 8 + dy:n * 8 + dy + 8, dx:dx + SP],
                                    start=(i == 0), stop=(i == 17),
                                )
                                i += 1
                        # prefetch next sample's weights + spmap broadcasts
                        if m == 0 and n == 6 and b + 1 < BPC:
                            load_wsc(b + 1)
                            load_smb(b + 1)
                        # prefetch next sample's image in m=1
                        if m == 1 and 1 <= n <= 3 and b + 1 < BPC:
                            if not prefetched:
                                xpn = prefetch_xp(b + 1)
                                prefetched = True
                            load_band(xpn, b + 1, n - 1, 0)
                            load_band(xpn, b + 1, n - 1, 1)
                        # epilogue: out = psum * spmap[yx] (all scales folded
                        # into wsc on host)
                        ot = opool.tile([128, 512], F32,
                                        name=f"ot_{b}_{m}_{n}", tag="ot")
                        nc.vector.tensor_tensor(
                            out=ot, in0=ps,
                            in1=smb_tiles[b][n], op=ALU.mult,
                        )
                        nc.sync.dma_start(
                            out=out_d.ap()[b, m * 128:(m + 1) * 128, n * 8:(n + 1) * 8, :],
                            in_=ot.rearrange("p (r c) -> p r c", c=SP),
                        )
            _stack.close()
    return nc


_prog_cache = {}


def _get_program() -> bass.Bass:
    if "nc" not in _prog_cache:
        _prog_cache["nc"] = _build_program()
    return _prog_cache["nc"]


def _make_in_maps(inputs):
    x = np.asarray(inputs["x"], dtype=np.float32)
    x = np.pad(x, ((0, 0), (0, 0), (1, 1), (1, 1))).astype(ml_dtypes.bfloat16)
    style_in = np.asarray(inputs["style_in"], dtype=np.float64)
    weight = np.asarray(inputs["weight"], dtype=np.float64)
    mod_w = np.asarray(inputs["mod_w"], dtype=np.float64)
    mod_b = np.asarray(inputs["mod_b"], dtype=np.float64)
    sp_w = np.asarray(inputs["sp_w"], dtype=np.float64)
    sp_b = np.asarray(inputs["sp_b"], dtype=np.float64)

    # ---- host-side small math (float64, a few MFLOPs total) ----
    style_chan, style_spatial = style_in[:, :256], style_in[:, 256:]
    style = style_chan @ (mod_w * LS).T + mod_b                 # [B, CIN]
    # demod via S2q[co,ci] = sum_kk weight^2 (exact same sum as reference)
    w0 = weight[0]                                              # [COUT,CIN,3,3]
    s2q = np.sum(w0 * w0, axis=(2, 3))                          # [COUT, CIN]
    demodsq = (CS * CS) * (style * style) @ s2q.T               # [B, COUT]
    demod = 1.0 / np.sqrt(demodsq + EPS)
    sp = style_spatial @ (sp_w * LS).T + sp_b                   # [B, YX]
    demod_sp = np.sqrt(YX / np.sum(sp * sp, axis=1) + EPS)      # [B]
    dcol = CS * demod * demod_sp[:, None]                       # [B, COUT]
    spmd = sp.astype(ml_dtypes.bfloat16)                        # [B, YX]

    # per-sample folded weights: [B, CIN, COUT*KK] bf16
    wT = np.ascontiguousarray(w0.transpose(1, 0, 2, 3)).reshape(CIN, COUT, K * K)
    wsc = (wT[None].astype(np.float32)
           * style.astype(np.float32)[:, :, None, None]
           * dcol.astype(np.float32)[:, None, :, None])         # [B,CIN,COUT,KK]
    wsc = wsc.reshape(B, CIN, CKK).astype(ml_dtypes.bfloat16)

    in_maps = []
    for c in range(N_CORES):
        sl = slice(c * BPC, (c + 1) * BPC)
        in_maps.append({
            "x": np.ascontiguousarray(x[sl]),
            "wsc": np.ascontiguousarray(wsc[sl]),
            "spmd": np.ascontiguousarray(spmd[sl]),
        })
    return in_maps


def _run(inputs, trace=False):
    nc = _get_program()
    in_maps = _make_in_maps(inputs)
    res = run_bass_kernel_spmd(nc, in_maps, core_ids=list(range(N_CORES)), trace=trace)
    out = np.concatenate([res.results[c]["out"] for c in range(N_CORES)], axis=0)
    return out, res


def kernel(**inputs) -> np.ndarray:
    out, _ = _run(inputs, trace=False)
    return out
